# revision 12
# baseline (speedup 1.0000x reference)
"""PointGroup clusters_voxelization kernel for Trainium2 (8 NeuronCores).

Strategy: shard the 1024 clusters across 8 cores (128 each), feats/coords
replicated via a packed fp16 table.  Per core the work runs in 8 rounds of
16 clusters; each cluster occupies 8 SBUF partitions (256 points each), so
a whole round (32768 points) is SBUF-resident at once:

  1. batched indirect gathers (128 rows / instruction = 16384 descriptors)
     pull fp16 table rows into a padded [128, 256, 36] tile — 2 gather
     instructions per round instead of 256, amortizing the ~1us SWDGE
     fixed overhead that dominated the naive one-row-per-slot scheme
  2. per-partition sum/min/max over the strided coord lanes, then a tiny
     SBUF->SBUF DMA regroups the 8 partial stats of each cluster onto one
     partition, lane-blocked so each stat reduces over 8 contiguous lanes
  3. per-cluster scale/offset params on 16 partitions, folded to the
     2-scalar form  out = raw * s + b,  broadcast back to 128 partitions
     with a 0-stride DMA
  4. pack: feats lanes cast fp16->f32 into a contiguous [128, 256, 35]
     tile while the coord lanes get the fused  *s + b  transform, then
     one large-descriptor DMA (35840B/partition) writes the final rows —
     no second pass over the output

fp16 table halves the gather traffic (the random 140B-row gather was the
single largest DMA cost); quantization error ~1e-4 relative, far under
the 2e-2 gate.
"""
import numpy as np

import concourse.bass as bass
import concourse.bacc as bacc
import concourse.tile as tile
import concourse.mybir as mybir
from concourse import bass_utils

N = 1048576
C = 32
NCLUSTER = 1024
PTS = 2048
S = NCLUSTER * PTS
NCORES = 8
P = 128                      # SBUF partitions
PPC = S // NCORES            # points per core = 262144
ROW = C + 3                  # 35 values per row
ROWP = ROW + 1               # padded SBUF row (keeps gather descriptors 70B)
RNDS = 16                    # rounds per core
GC = 8                       # clusters per round
QP = P // GC                 # partitions per cluster = 16
PP = PTS // QP               # points per partition per round = 128
GATH = 1                     # gather instructions per round
GROWS = PP // GATH           # rows per gather instruction = 128

_CACHE = {}


def _build_program(fullscale: float, scale: float):
    key = (fullscale, scale)
    if key in _CACHE:
        return _CACHE[key]

    fs = float(fullscale)
    sc = float(scale)
    f32 = mybir.dt.float32
    f16 = mybir.dt.float16

    nc = bacc.Bacc("TRN2", target_bir_lowering=False, debug=False)
    table_d = nc.dram_tensor("table", (N, ROW), f16, kind="ExternalInput")
    pid_d = nc.dram_tensor("pid", (PPC,), mybir.dt.int32, kind="ExternalInput")
    jit_d = nc.dram_tensor("jit", (2, 3), f32, kind="ExternalInput")
    out_d = nc.dram_tensor("out", (PPC, ROW), f32, kind="ExternalOutput")

    with tile.TileContext(nc) as tc:
        with (
            tc.tile_pool(name="one", bufs=1) as one,
            tc.tile_pool(name="gat", bufs=6) as gat,
            tc.tile_pool(name="pck", bufs=3) as pck,
            tc.tile_pool(name="sm", bufs=4) as smp,
        ):
            # point ids, laid out so partition p of round r covers the PP
            # consecutive points starting at P*PP*r + PP*p.  Round 0's slab
            # loads first so the first gather isn't gated on the full table.
            idx_t = one.tile([P, RNDS * PP], mybir.dt.int32)
            nc.sync.dma_start(
                out=idx_t[:, 0:PP],
                in_=bass.AP(tensor=pid_d, offset=0, ap=[[PP, P], [1, PP]]),
            )
            nc.sync.dma_start(
                out=idx_t[:, PP:],
                in_=bass.AP(
                    tensor=pid_d, offset=P * PP,
                    ap=[[PP, P], [P * PP, RNDS - 1], [1, PP]],
                ),
            )
            jit_t = one.tile([P, 6], f32)
            jsrc = jit_d.ap().rearrange("a b -> (a b)")
            nc.sync.dma_start(
                out=jit_t[:],
                in_=bass.AP(tensor=jsrc.tensor, offset=jsrc.offset,
                            ap=[[0, P]] + jsrc.ap),
            )

            # Stage-shifted issue: at iteration it we issue
            #   gather+stats for round it, combine/params for it-1,
            #   transform+write for it-2
            # so the in-order DVE queue never head-blocks on the regroup /
            # broadcast DMA semaphores of the round it is about to finish.
            asm_t = {}
            stg_t = {}
            prmb_t = {}

            def issue_gather_stats(r):
                asm = gat.tile([P, PP, ROWP], f16, name="asm")
                asm_t[r] = asm
                # round 0 splits its gather so the first descriptor batch
                # reaches the DMA engines sooner
                nsplit = 2 if r == 0 else GATH
                rows = PP // nsplit
                for h in range(nsplit):
                    lo = r * PP + h * rows
                    nc.gpsimd.indirect_dma_start(
                        out=asm[:, h * rows : (h + 1) * rows, 0:ROW],
                        out_offset=None,
                        in_=table_d.ap(),
                        in_offset=bass.IndirectOffsetOnAxis(
                            ap=idx_t[:, lo : lo + rows], axis=0
                        ),
                    )
                # per-partition coord stats (sum/min/max x 3 comps)
                st = smp.tile([P, 12], f32, name="st")
                for c in range(3):
                    nc.vector.reduce_sum(
                        out=st[:, c : c + 1], in_=asm[:, :, C + c],
                        axis=mybir.AxisListType.X,
                    )
                    nc.vector.tensor_reduce(
                        out=st[:, 3 + c : 4 + c], in_=asm[:, :, C + c],
                        axis=mybir.AxisListType.X, op=mybir.AluOpType.min,
                    )
                    nc.vector.reduce_max(
                        out=st[:, 6 + c : 7 + c], in_=asm[:, :, C + c],
                        axis=mybir.AxisListType.X,
                    )
                # regroup: cluster c's QP partial stat rows land on partition
                # c as QP consecutive 9-lane blocks (plain flatten)
                stg = smp.tile([GC, 9 * QP], f32, name="stg")
                stg_t[r] = stg
                nc.sync.dma_start(out=stg[:], in_=st[:, 0:9])

            def issue_params(r):
                stg_ap = stg_t[r][:]
                # combine across the QP blocks: stat j is at lanes j, j+9, ...
                red = smp.tile([GC, 12], f32, name="red")
                for i, op in enumerate(
                    (mybir.AluOpType.add, mybir.AluOpType.min, mybir.AluOpType.max)
                ):
                    nc.vector.tensor_reduce(
                        out=red[:, 3 * i : 3 * i + 3],
                        in_=bass.AP(tensor=stg_ap.tensor,
                                    offset=stg_ap.offset + 3 * i,
                                    ap=[stg_ap.ap[0], [1, 3], [9, QP]]),
                        axis=mybir.AxisListType.X, op=op,
                    )
                # per-cluster params on GC partitions, folded to
                # out = raw * s + b with b = off - cmean*s
                pr = smp.tile([GC, 24], f32, name="pr")
                CM, WD, MN, T0, T1, OFF = (
                    slice(0, 3), slice(3, 6), slice(6, 9),
                    slice(9, 12), slice(12, 15), slice(15, 18),
                )
                sc_t = smp.tile([GC, 4], f32, name="sc_t")
                nc.vector.tensor_scalar_mul(pr[:, CM], red[:, 0:3], 1.0 / PTS)
                nc.vector.tensor_tensor(
                    out=pr[:, WD], in0=red[:, 6:9], in1=red[:, 3:6],
                    op=mybir.AluOpType.subtract,
                )
                nc.vector.reduce_max(
                    out=sc_t[:, 0:1], in_=pr[:, WD], axis=mybir.AxisListType.X
                )
                # s = min(fs/wmax - 0.01, scale) via IEEE reciprocal
                nc.vector.reciprocal(out=sc_t[:, 1:2], in_=sc_t[:, 0:1])
                nc.vector.tensor_scalar(
                    out=sc_t[:, 2:3], in0=sc_t[:, 1:2], scalar1=fs, scalar2=-0.01,
                    op0=mybir.AluOpType.mult, op1=mybir.AluOpType.add,
                )
                nc.vector.tensor_scalar(
                    out=sc_t[:, 2:3], in0=sc_t[:, 2:3], scalar1=sc, scalar2=None,
                    op0=mybir.AluOpType.min,
                )
                s_ap = sc_t[:, 2:3]
                # mn = (cmin - cmean) * s   (cmin arrives uncentered)
                nc.vector.tensor_tensor(
                    out=pr[:, MN], in0=red[:, 3:6], in1=pr[:, CM],
                    op=mybir.AluOpType.subtract,
                )
                nc.vector.tensor_scalar(
                    out=pr[:, MN], in0=pr[:, MN], scalar1=s_ap, scalar2=None,
                    op0=mybir.AluOpType.mult,
                )
                # t = fs - wd*s ; t0 = max(t-.001, 0) ; t1 = min(t+.001, 0)
                nc.vector.tensor_scalar(
                    out=pr[:, T0], in0=pr[:, WD], scalar1=s_ap, scalar2=None,
                    op0=mybir.AluOpType.mult,
                )
                nc.vector.tensor_scalar(
                    out=pr[:, T0], in0=pr[:, T0], scalar1=-1.0, scalar2=fs,
                    op0=mybir.AluOpType.mult, op1=mybir.AluOpType.add,
                )
                nc.vector.tensor_scalar(
                    out=pr[:, T1], in0=pr[:, T0], scalar1=0.001, scalar2=0.0,
                    op0=mybir.AluOpType.add, op1=mybir.AluOpType.min,
                )
                nc.vector.tensor_scalar(
                    out=pr[:, T0], in0=pr[:, T0], scalar1=-0.001, scalar2=0.0,
                    op0=mybir.AluOpType.add, op1=mybir.AluOpType.max,
                )
                # off = t0*j0 - mn + t1*j1 ; b = off - cmean*s
                nc.vector.tensor_tensor(
                    out=pr[:, T0], in0=pr[:, T0], in1=jit_t[0:GC, 0:3],
                    op=mybir.AluOpType.mult,
                )
                nc.vector.tensor_tensor(
                    out=pr[:, T1], in0=pr[:, T1], in1=jit_t[0:GC, 3:6],
                    op=mybir.AluOpType.mult,
                )
                nc.vector.tensor_tensor(
                    out=pr[:, OFF], in0=pr[:, T0], in1=pr[:, MN],
                    op=mybir.AluOpType.subtract,
                )
                nc.vector.tensor_tensor(
                    out=pr[:, OFF], in0=pr[:, OFF], in1=pr[:, T1],
                    op=mybir.AluOpType.add,
                )
                prm = smp.tile([GC, 4], f32, name="prm")
                nc.vector.tensor_copy(out=prm[:, 0:1], in_=s_ap)
                nc.vector.tensor_scalar(
                    out=pr[:, CM], in0=pr[:, CM], scalar1=s_ap, scalar2=None,
                    op0=mybir.AluOpType.mult,
                )
                nc.vector.tensor_tensor(
                    out=prm[:, 1:4], in0=pr[:, OFF], in1=pr[:, CM],
                    op=mybir.AluOpType.subtract,
                )
                # broadcast [s, b0, b1, b2] to the QP partitions of each
                # cluster via 0-stride re-read
                prmb = smp.tile([P, 4], f32, name="prmb")
                prmb_t[r] = prmb
                prm_ap = prm[:]
                nc.sync.dma_start(
                    out=prmb[:],
                    in_=bass.AP(tensor=prm_ap.tensor, offset=prm_ap.offset,
                                ap=[prm_ap.ap[0], [0, QP], [1, 4]]),
                )

            def issue_transform_write(r):
                asm = asm_t.pop(r)
                prmb = prmb_t.pop(r)
                stg_t.pop(r, None)
                # pack + transform into contiguous f32 rows; the feats cast
                # runs on the otherwise-idle Activation engine
                pk = pck.tile([P, PP, ROW], f32, name="pk")
                nc.scalar.copy(out=pk[:, :, 0:C], in_=asm[:, :, 0:C])
                for c in range(3):
                    nc.vector.tensor_scalar(
                        out=pk[:, :, C + c], in0=asm[:, :, C + c],
                        scalar1=prmb[:, 0:1], scalar2=prmb[:, 1 + c : 2 + c],
                        op0=mybir.AluOpType.mult, op1=mybir.AluOpType.add,
                    )
                # one large-descriptor write of the round's final rows,
                # issued from the Activation queue so it never head-blocks
                # the SP queue's stats DMAs
                nc.scalar.dma_start(
                    out=bass.AP(tensor=out_d, offset=r * P * PP * ROW,
                                ap=[[PP * ROW, P], [1, PP * ROW]]),
                    in_=pk[:],
                )

            for it in range(RNDS + 2):
                if it < RNDS:
                    issue_gather_stats(it)
                if 0 <= it - 1 < RNDS:
                    issue_params(it - 1)
                if 0 <= it - 2 < RNDS:
                    issue_transform_write(it - 2)

    nc.compile()
    _CACHE[key] = nc
    return nc


def _reference_numpy(clusters_idx, clusters_offset, feats, coords, jitter, fullscale, scale):
    seg = clusters_idx[:, 0].astype(np.int64)
    pid = clusters_idx[:, 1].astype(np.int64)
    nC = clusters_offset.shape[0] - 1
    fs = np.float32(fullscale)
    cf = feats[pid]
    cc = coords[pid].astype(np.float32)
    cnt = np.diff(clusters_offset).astype(np.float32)[:, None]
    sums = np.zeros((nC, 3), np.float32)
    np.add.at(sums, seg, cc)
    cmean = sums / np.maximum(cnt, 1.0)
    ccc = cc - cmean[seg]
    cmin = np.full((nC, 3), np.inf, np.float32)
    cmax = np.full((nC, 3), -np.inf, np.float32)
    np.minimum.at(cmin, seg, ccc)
    np.maximum.at(cmax, seg, ccc)
    cscale = 1.0 / ((cmax - cmin) / fs).max(axis=1) - np.float32(0.01)
    cscale = np.minimum(cscale, np.float32(scale)).astype(np.float32)
    mn = cmin * cscale[:, None]
    mx = cmax * cscale[:, None]
    ccc = ccc * cscale[seg][:, None]
    rng = mx - mn
    off = (-mn + np.maximum(fs - rng - 0.001, 0.0) * jitter[0]
           + np.minimum(fs - rng + 0.001, 0.0) * jitter[1]).astype(np.float32)
    ccc = ccc + off[seg]
    return np.concatenate([cf, ccc], axis=1).astype(np.float32)


def _make_in_maps(clusters_idx, feats, coords, jitter):
    table = np.ascontiguousarray(
        np.concatenate([feats, coords], axis=1).astype(np.float16)
    )
    pid_full = np.ascontiguousarray(clusters_idx[:, 1].astype(np.int32))
    in_maps = []
    for k in range(NCORES):
        in_maps.append(
            {
                "table": table,
                "pid": pid_full[k * PPC : (k + 1) * PPC],
                "jit": jitter,
            }
        )
    return in_maps


def kernel(clusters_idx, clusters_offset, feats, coords, jitter, fullscale, scale):
    clusters_idx = np.asarray(clusters_idx)
    clusters_offset = np.asarray(clusters_offset)
    feats = np.asarray(feats, dtype=np.float32)
    coords = np.asarray(coords, dtype=np.float32)
    jitter = np.asarray(jitter, dtype=np.float32)

    fs = float(np.asarray(fullscale).item()) if not isinstance(fullscale, (int, float)) else float(fullscale)
    sc = float(np.asarray(scale).item()) if not isinstance(scale, (int, float)) else float(scale)

    uniform = (
        clusters_idx.shape == (S, 2)
        and clusters_offset.shape == (NCLUSTER + 1,)
        and feats.shape == (N, C)
        and coords.shape == (N, 3)
        and np.array_equal(
            clusters_offset,
            np.arange(NCLUSTER + 1, dtype=np.int64) * PTS,
        )
        and np.array_equal(
            clusters_idx[:, 0],
            np.repeat(np.arange(NCLUSTER, dtype=np.int64), PTS),
        )
    )
    if not uniform:
        return _reference_numpy(
            clusters_idx, clusters_offset, feats, coords, jitter, fs, sc
        )

    nc = _build_program(fs, sc)
    in_maps = _make_in_maps(clusters_idx, feats, coords, jitter)
    res = bass_utils.run_bass_kernel_spmd(nc, in_maps, core_ids=list(range(NCORES)))
    return np.concatenate([res.results[k]["out"] for k in range(NCORES)], axis=0)
